# revision 8
# baseline (speedup 1.0000x reference)
"""Trainium2 Bass kernel for nn_AttentionBlock (B=4, S=1024, E=1024, H=16, FF=4096).

Sharding: 8-way data-parallel over (batch, seq-half) tokens. Each core owns 512
query tokens of one batch; it computes K/V for the batch's full 1024 tokens
locally (cheap redundant compute instead of a collective). All matmuls run in
bf16 on the TensorEngine (f32 PSUM accumulation); softmax, residuals and
layernorms stay in f32.

SBUF pool lifetimes are two LIFO stacks:
  left : always | p_qkv(P0-P3) | p_xT(P0-P1) | phase transients | p_h(P4-end) |
         p_x1T(P4-P5a) | ...
  right: p_xog(P0-P4) | p_ctx(P3-P4) || p_ffT(P5a-P5b)
"""

import sys

sys.path.insert(0, "/opt/trn_rl_repo")

import numpy as np

B, S, E = 4, 1024, 1024
H, DK, HS = 16, 64, 64
FF = 4096
EPS = 1e-6
N_CORES = 8
T_OWN = 512  # tokens owned per core
T_FULL = 1024  # tokens of the core's batch (for K/V)
P = 128

_CACHE = {}


def _build_nc():
    import concourse.bass as bass  # noqa: F401
    import concourse.mybir as mybir
    import concourse.tile as tile
    from concourse import bacc
    from concourse.masks import make_identity

    F32 = mybir.dt.float32
    BF16 = mybir.dt.bfloat16
    AX = mybir.AxisListType
    AF = mybir.ActivationFunctionType
    OP = mybir.AluOpType

    nc = bacc.Bacc(None, target_bir_lowering=False, debug=False)

    x_d = nc.declare_dram_parameter("x", [T_FULL, E], F32, isOutput=False)
    pos_d = nc.declare_dram_parameter("pos", [T_OWN, S * 8], F32, isOutput=False)
    wq_d = nc.declare_dram_parameter("wq", [E, H * DK], BF16, isOutput=False)
    wk_d = nc.declare_dram_parameter("wk", [E, H * DK], BF16, isOutput=False)
    wv_d = nc.declare_dram_parameter("wv", [E, H * HS], BF16, isOutput=False)
    wg_d = nc.declare_dram_parameter("wg", [E, E], BF16, isOutput=False)
    wo_d = nc.declare_dram_parameter("wo", [H * HS, E], BF16, isOutput=False)
    wf1_d = nc.declare_dram_parameter("wf1", [E, FF], BF16, isOutput=False)
    wfg_d = nc.declare_dram_parameter("wfg", [E, FF], BF16, isOutput=False)
    wf2_d = nc.declare_dram_parameter("wf2", [FF, E], BF16, isOutput=False)
    bq_d = nc.declare_dram_parameter("bq", [H * DK], F32, isOutput=False)
    bk_d = nc.declare_dram_parameter("bk", [H * DK], F32, isOutput=False)
    brows_d = nc.declare_dram_parameter("brows", [4 * E], BF16, isOutput=False)
    bf1_d = nc.declare_dram_parameter("bf1", [FF], F32, isOutput=False)
    bfg_d = nc.declare_dram_parameter("bfg", [FF], F32, isOutput=False)
    g1_d = nc.declare_dram_parameter("g1", [E], F32, isOutput=False)
    b1_d = nc.declare_dram_parameter("b1", [E], F32, isOutput=False)
    g2_d = nc.declare_dram_parameter("g2", [E], F32, isOutput=False)
    b2_d = nc.declare_dram_parameter("b2", [E], F32, isOutput=False)
    out_d = nc.declare_dram_parameter("out", [T_OWN, E], F32, isOutput=True)

    ET = E // P  # 8 e-tiles
    TT = T_FULL // P  # 8 token tiles (full)
    TO = T_OWN // P  # 4 own token tiles
    FT = FF // P  # 32 ff tiles

    with tile.TileContext(nc) as tc:
        # ---- whole-kernel pool: constants, biases, small stats -------------
        always = tc.alloc_tile_pool(name="always", bufs=1)

        id_f32 = always.tile([P, P], F32)
        make_identity(nc, id_f32)
        id_bf = always.tile([P, P], BF16)
        make_identity(nc, id_bf)
        ones_row = always.tile([1, P], BF16)
        nc.gpsimd.memset(ones_row[:, :], 1.0)
        eps_col = always.tile([P, 1], F32)
        nc.gpsimd.memset(eps_col[:, :], EPS)

        bq_sb = always.tile([P, ET], F32)
        nc.sync.dma_start(out=bq_sb[:, :], in_=bq_d[:].rearrange("(a p) -> p a", p=P))
        bk_sb = always.tile([P, ET], F32)
        nc.sync.dma_start(out=bk_sb[:, :], in_=bk_d[:].rearrange("(a p) -> p a", p=P))
        bf1_sb = always.tile([P, FT], F32)
        nc.sync.dma_start(out=bf1_sb[:, :], in_=bf1_d[:].rearrange("(a p) -> p a", p=P))
        bfg_sb = always.tile([P, FT], F32)
        nc.sync.dma_start(out=bfg_sb[:, :], in_=bfg_d[:].rearrange("(a p) -> p a", p=P))

        brow_bf = always.tile([1, 4 * E], BF16)
        nc.sync.dma_start(out=brow_bf[:, :], in_=brows_d[None, :])
        bv_row = brow_bf[:, 0:E]
        bg_row = brow_bf[:, E : 2 * E]
        bo_row = brow_bf[:, 2 * E : 3 * E]
        bf2_row = brow_bf[:, 3 * E : 4 * E]

        ln_bc = always.tile([P, 4 * E], F32)
        ones_col_f32 = always.tile([1, P], F32)
        nc.gpsimd.memset(ones_col_f32[:, :], 1.0)
        p_lnrow = tc.alloc_tile_pool(name="p_lnrow", bufs=1)
        p_lnb_psum = tc.alloc_tile_pool(name="p_lnb_psum", bufs=2, space="PSUM")
        ln_row = p_lnrow.tile([1, 4 * E], F32)
        nc.sync.dma_start(out=ln_row[:, 0:E], in_=g1_d[None, :])
        nc.sync.dma_start(out=ln_row[:, E : 2 * E], in_=b1_d[None, :])
        nc.sync.dma_start(out=ln_row[:, 2 * E : 3 * E], in_=g2_d[None, :])
        nc.sync.dma_start(out=ln_row[:, 3 * E : 4 * E], in_=b2_d[None, :])
        # broadcast ln rows to all partitions via K=1 matmul (gpsimd
        # partition_broadcast needs the non-default ucode library)
        for c8 in range(8):
            psb = p_lnb_psum.tile([P, 512], F32, tag="lnb", name="lnb")
            nc.tensor.matmul(
                psb[:, :],
                lhsT=ones_col_f32[:, :],
                rhs=ln_row[:, c8 * 512 : (c8 + 1) * 512],
                start=True,
                stop=True,
            )
            nc.scalar.activation(ln_bc[:, c8 * 512 : (c8 + 1) * 512], psb[:, :], AF.Copy)
        p_lnb_psum.release()
        p_lnrow.release()
        g1_bc = ln_bc[:, 0:E]
        b1_bc = ln_bc[:, E : 2 * E]
        g2_bc = ln_bc[:, 2 * E : 3 * E]
        b2_bc = ln_bc[:, 3 * E : 4 * E]

        # ---- persistent activations (lifetimes per stack plan) -------------
        p_xog = tc.alloc_tile_pool(name="p_xog", bufs=1, side="right")
        x_own = p_xog.tile([P, TO, E], F32)
        gate_sb = p_xog.tile([P, TO, E], F32)

        p_qkv = tc.alloc_tile_pool(name="p_qkv", bufs=1)
        q_T = p_qkv.tile([P, ET, T_OWN], BF16)
        k_T = p_qkv.tile([P, ET, T_FULL], BF16)
        v_aug = p_qkv.tile([P, TT, H * (HS + 1)], BF16)
        pos_T = p_qkv.tile([P, TT, T_OWN], BF16)
        v_aug4 = v_aug.rearrange("p t (h c) -> p t h c", c=HS + 1)

        p_xT = tc.alloc_tile_pool(name="p_xT", bufs=1)
        x_T = p_xT.tile([P, ET, T_FULL], BF16)

        nc.sync.dma_start(
            out=x_own[:, :, :],
            in_=x_d[0:T_OWN, :].rearrange("(t p) e -> p t e", p=P),
        )

        # ================= P0: load x, transpose to feature-major ==========
        p0_stage = tc.alloc_tile_pool(name="p0_stage", bufs=1)
        p0_psum = tc.alloc_tile_pool(name="p0_psum", bufs=2, space="PSUM")
        x_stage = p0_stage.tile([P, TT, E], F32)
        for t in range(TT):
            nc.sync.dma_start(out=x_stage[:, t, :], in_=x_d[t * P : (t + 1) * P, :])
        for et in range(ET):
            for tg in range(2):
                ps = p0_psum.tile([P, 512], F32, tag="tp", name="tp")
                for i in range(4):
                    t = tg * 4 + i
                    nc.tensor.transpose(
                        ps[:, i * P : (i + 1) * P],
                        x_stage[:, t, et * P : (et + 1) * P],
                        id_f32[:, :],
                    )
                nc.scalar.activation(
                    x_T[:, et, tg * 512 : (tg + 1) * 512], ps[:, :], AF.Copy
                )
        p0_psum.release()
        p0_stage.release()

        # ================= P1: QKV + gate projections =======================
        p1_w = tc.alloc_tile_pool(name="p1_w", bufs=2)
        p1_psum = tc.alloc_tile_pool(name="p1_psum", bufs=4, space="PSUM")
        wq_sb = p1_w.tile([P, ET, H * DK], BF16, tag="w", name="wq_sb")
        wk_sb = p1_w.tile([P, ET, H * DK], BF16, tag="w", name="wk_sb")
        wv_sb = p1_w.tile([P, ET, H * HS], BF16, tag="w", name="wv_sb")
        wg_sb = p1_w.tile([P, ET, E], BF16, tag="w", name="wg_sb")
        for et in range(ET):
            nc.sync.dma_start(out=wq_sb[:, et, :], in_=wq_d[et * P : (et + 1) * P, :])
        for et in range(ET):
            nc.sync.dma_start(out=wk_sb[:, et, :], in_=wk_d[et * P : (et + 1) * P, :])
        for et in range(ET):
            nc.sync.dma_start(out=wv_sb[:, et, :], in_=wv_d[et * P : (et + 1) * P, :])
        for et in range(ET):
            nc.sync.dma_start(out=wg_sb[:, et, :], in_=wg_d[et * P : (et + 1) * P, :])

        # q_T[nt] = (x[own] @ Wq)_T   (feature-major out via lhsT=W)
        for nt in range(ET):
            ps = p1_psum.tile([P, 512], F32, tag="qk", name="psq")
            for et in range(ET):
                nc.tensor.matmul(
                    ps[:, :],
                    lhsT=wq_sb[:, et, nt * P : (nt + 1) * P],
                    rhs=x_T[:, et, 0:T_OWN],
                    start=(et == 0),
                    stop=(et == ET - 1),
                )
            nc.scalar.activation(
                q_T[:, nt, :], ps[:, :], AF.Identity, bias=bq_sb[:, nt : nt + 1]
            )
        # k_T[nt] over full 1024 tokens
        for nt in range(ET):
            for ch in range(2):
                ps = p1_psum.tile([P, 512], F32, tag="qk", name="psk")
                for et in range(ET):
                    nc.tensor.matmul(
                        ps[:, :],
                        lhsT=wk_sb[:, et, nt * P : (nt + 1) * P],
                        rhs=x_T[:, et, ch * 512 : (ch + 1) * 512],
                        start=(et == 0),
                        stop=(et == ET - 1),
                    )
                nc.scalar.activation(
                    k_T[:, nt, ch * 512 : (ch + 1) * 512],
                    ps[:, :],
                    AF.Identity,
                    bias=bk_sb[:, nt : nt + 1],
                )
        # v token-major over full tokens (+ bias row); scatter per head
        for t in range(TT):
            for hg in range(2):
                ps = p1_psum.tile([P, 512], F32, tag="qk", name="psv")
                for et in range(ET):
                    nc.tensor.matmul(
                        ps[:, :],
                        lhsT=x_T[:, et, t * P : (t + 1) * P],
                        rhs=wv_sb[:, et, hg * 512 : (hg + 1) * 512],
                        start=(et == 0),
                        stop=False,
                    )
                nc.tensor.matmul(
                    ps[:, :],
                    lhsT=ones_row[:, :],
                    rhs=bv_row[:, hg * 512 : (hg + 1) * 512],
                    start=False,
                    stop=True,
                )
                nc.vector.tensor_copy(
                    v_aug4[:, t, hg * 8 : (hg + 1) * 8, 0:HS],
                    ps[:, :].rearrange("p (h c) -> p h c", c=HS),
                )
        nc.gpsimd.memset(v_aug4[:, :, :, HS : HS + 1], 1.0)
        # gate = sigmoid(x @ Wg + bg), token-major, own tokens
        for t in range(TO):
            for eg in range(2):
                ps = p1_psum.tile([P, 512], F32, tag="qk", name="psg")
                for et in range(ET):
                    nc.tensor.matmul(
                        ps[:, :],
                        lhsT=x_T[:, et, t * P : (t + 1) * P],
                        rhs=wg_sb[:, et, eg * 512 : (eg + 1) * 512],
                        start=(et == 0),
                        stop=False,
                    )
                nc.tensor.matmul(
                    ps[:, :],
                    lhsT=ones_row[:, :],
                    rhs=bg_row[:, eg * 512 : (eg + 1) * 512],
                    start=False,
                    stop=True,
                )
                nc.scalar.activation(
                    gate_sb[:, t, eg * 512 : (eg + 1) * 512], ps[:, :], AF.Sigmoid
                )
        p1_psum.release()
        p1_w.release()
        p_xT.release()

        # ================= P2: positional bias ==============================
        p2_stage = tc.alloc_tile_pool(name="p2_stage", bufs=3)
        p2_tok = tc.alloc_tile_pool(name="p2_tok", bufs=1)
        p2_psum = tc.alloc_tile_pool(name="p2_psum", bufs=2, space="PSUM")
        pos_tok = p2_tok.tile([P, TO, S], F32)
        for qt in range(TO):
            for kc in range(4):
                stage = p2_stage.tile([P, 2048], F32, tag="pos", name="poss")
                nc.sync.dma_start(
                    out=stage[:, :],
                    in_=pos_d[qt * P : (qt + 1) * P, kc * 2048 : (kc + 1) * 2048],
                )
                nc.vector.tensor_reduce(
                    pos_tok[:, qt, kc * 256 : (kc + 1) * 256],
                    stage.rearrange("p (k c) -> p k c", c=8),
                    axis=AX.X,
                    op=OP.add,
                )
        for kt in range(TT):
            ps = p2_psum.tile([P, 512], F32, tag="ptp", name="ptps")
            for qt in range(TO):
                nc.tensor.transpose(
                    ps[:, qt * P : (qt + 1) * P],
                    pos_tok[:, qt, kt * P : (kt + 1) * P],
                    id_f32[:, :],
                )
            nc.scalar.activation(pos_T[:, kt, :], ps[:, :], AF.Copy)
        p2_psum.release()
        p2_tok.release()
        p2_stage.release()

        # ================= P3: attention, one head at a time ================
        p_ctx = tc.alloc_tile_pool(name="p_ctx", bufs=1, side="right")
        ctx_T = p_ctx.tile([P, ET, T_OWN], BF16)
        p3_r32 = tc.alloc_tile_pool(name="p3_r32", bufs=2)
        p3_rbf = tc.alloc_tile_pool(name="p3_rbf", bufs=3)
        p3_p = tc.alloc_tile_pool(name="p3_p", bufs=2)
        p3_rfsb = tc.alloc_tile_pool(name="p3_rfsb", bufs=2)
        p3_sc = tc.alloc_tile_pool(name="p3_sc", bufs=3, space="PSUM")
        p3_pv = tc.alloc_tile_pool(name="p3_pv", bufs=3, space="PSUM")
        p3_rf = tc.alloc_tile_pool(name="p3_rf", bufs=2, space="PSUM")
        pv_prev = None
        rbf_prev = None
        for h in range(H):
            hp = h % 2
            nt = h // 2
            pt = p3_p.tile([P, TT, 512], BF16, tag="pt", name="pt")
            pv = p3_pv.tile([P, 512], F32, tag="pv", name="pv")
            for kt in range(TT):
                sc = p3_sc.tile([P, 512], F32, tag="sc", name="sc")
                nc.tensor.matmul(
                    sc[:, :],
                    lhsT=k_T[hp * 64 : hp * 64 + 64, nt, kt * P : (kt + 1) * P],
                    rhs=q_T[hp * 64 : hp * 64 + 64, nt, :],
                    start=True,
                    stop=False,
                )
                nc.tensor.matmul(
                    sc[:, :], lhsT=id_bf[:, :], rhs=pos_T[:, kt, :], start=False, stop=True
                )
                nc.scalar.activation(pt[:, kt, :], sc[:, :], AF.Exp, scale=0.125)
                nc.tensor.matmul(
                    pv[0 : HS + 1, :],
                    lhsT=v_aug4[:, kt, h, :],
                    rhs=pt[:, kt, :],
                    start=(kt == 0),
                    stop=(kt == TT - 1),
                )
            r32 = p3_r32.tile([1, T_OWN], F32, tag="r32", name="r32")
            rbf = p3_rbf.tile([1, T_OWN], BF16, tag="rbf", name="rbf")
            nc.vector.reciprocal(r32[:, :], pv[HS : HS + 1, :])
            nc.vector.tensor_copy(rbf[:, :], r32[:, :])
            if hp == 1:
                rf = p3_rf.tile([P, 512], F32, tag="rf", name="rf")
                nc.tensor.matmul(
                    rf[0:64, :],
                    lhsT=ones_row[:, 0:64],
                    rhs=rbf_prev[:, :],
                    start=True,
                    stop=True,
                )
                nc.tensor.matmul(
                    rf[64:128, :],
                    lhsT=ones_row[:, 0:64],
                    rhs=rbf[:, :],
                    start=True,
                    stop=True,
                )
                rf_sb = p3_rfsb.tile([P, 512], F32, tag="rfsb", name="rfsb")
                nc.vector.tensor_copy(rf_sb[:, :], rf[:, :])
                nc.vector.tensor_mul(ctx_T[0:64, nt, :], pv_prev[0:64, :], rf_sb[0:64, :])
                nc.vector.tensor_mul(ctx_T[64:128, nt, :], pv[0:64, :], rf_sb[64:128, :])
            pv_prev = pv
            rbf_prev = rbf
        p3_rf.release()
        p3_pv.release()
        p3_sc.release()
        p3_rfsb.release()
        p3_p.release()
        p3_rbf.release()
        p3_r32.release()
        p_qkv.release()

        # ================= P4: Wo + gating + LN1 ============================
        p_h = tc.alloc_tile_pool(name="p_h", bufs=1)
        h_sb = p_h.tile([P, TO, E], F32)
        x1_tok = p_h.tile([P, TO, E], F32)
        p_x1T = tc.alloc_tile_pool(name="p_x1T", bufs=1)
        x1_T = p_x1T.tile([P, ET, T_OWN], BF16)
        p4_w = tc.alloc_tile_pool(name="p4_w", bufs=1)
        p4_tmp = tc.alloc_tile_pool(name="p4_tmp", bufs=2)
        p4_stat = tc.alloc_tile_pool(name="p4_stat", bufs=4)
        p4_psum = tc.alloc_tile_pool(name="p4_psum", bufs=2, space="PSUM")
        wo_sb = p4_w.tile([P, ET, E], BF16)
        for et in range(ET):
            nc.sync.dma_start(out=wo_sb[:, et, :], in_=wo_d[et * P : (et + 1) * P, :])
        for t in range(TO):
            for eg in range(2):
                ps = p4_psum.tile([P, 512], F32, tag="o", name="pso")
                for hst in range(ET):
                    nc.tensor.matmul(
                        ps[:, :],
                        lhsT=ctx_T[:, hst, t * P : (t + 1) * P],
                        rhs=wo_sb[:, hst, eg * 512 : (eg + 1) * 512],
                        start=(hst == 0),
                        stop=False,
                    )
                nc.tensor.matmul(
                    ps[:, :],
                    lhsT=ones_row[:, :],
                    rhs=bo_row[:, eg * 512 : (eg + 1) * 512],
                    start=False,
                    stop=True,
                )
                tmp = p4_tmp.tile([P, 512], F32, tag="gat", name="gat")
                nc.vector.tensor_mul(
                    tmp[:, :], ps[:, :], gate_sb[:, t, eg * 512 : (eg + 1) * 512]
                )
                nc.vector.tensor_add(
                    h_sb[:, t, eg * 512 : (eg + 1) * 512],
                    tmp[:, :],
                    x_own[:, t, eg * 512 : (eg + 1) * 512],
                )

        def layernorm(src, dst, g_bc, b_bc, tmpp, statp):
            # dst = (src - mean) / sqrt(var + eps) * g + b, rows = tokens
            for t in range(TO):
                scr = tmpp.tile([P, E], F32, tag="scr", name="scr")
                ssum = statp.tile([P, 1], F32, tag="st", name="ssum")
                nc.scalar.activation(
                    scr[:, :], src[:, t, :], AF.Identity, accum_out=ssum[:, :]
                )
                scr2 = tmpp.tile([P, E], F32, tag="scr2", name="scr2")
                ssq = statp.tile([P, 1], F32, tag="st", name="ssq")
                nc.scalar.activation(
                    scr2[:, :], src[:, t, :], AF.Square, accum_out=ssq[:, :]
                )
                nmu = statp.tile([P, 1], F32, tag="st", name="nmu")
                nc.vector.tensor_scalar_mul(nmu[:, :], ssum[:, :], -1.0 / E)
                musq = statp.tile([P, 1], F32, tag="st", name="musq")
                nc.vector.tensor_mul(musq[:, :], nmu[:, :], nmu[:, :])
                var = statp.tile([P, 1], F32, tag="st", name="var")
                nc.vector.scalar_tensor_tensor(
                    out=var[:, :],
                    in0=ssq[:, :],
                    scalar=1.0 / E,
                    in1=musq[:, :],
                    op0=OP.mult,
                    op1=OP.subtract,
                )
                # rstd = exp(-0.5 * ln(var + eps)) — keeps ACT in the exp/ln set
                lnv = statp.tile([P, 1], F32, tag="st", name="lnv")
                nc.scalar.activation(lnv[:, :], var[:, :], AF.Ln, bias=eps_col[:, :])
                rstd = statp.tile([P, 1], F32, tag="st", name="rstd")
                nc.scalar.activation(rstd[:, :], lnv[:, :], AF.Exp, scale=-0.5)
                bprime = statp.tile([P, 1], F32, tag="st", name="bprime")
                nc.vector.tensor_mul(bprime[:, :], nmu[:, :], rstd[:, :])
                xc = tmpp.tile([P, E], F32, tag="xc", name="xc")
                nc.scalar.activation(
                    xc[:, :],
                    src[:, t, :],
                    AF.Identity,
                    bias=bprime[:, :],
                    scale=rstd[:, :],
                )
                tmp2 = tmpp.tile([P, E], F32, tag="xg", name="xg")
                nc.vector.tensor_mul(tmp2[:, :], xc[:, :], g_bc)
                nc.vector.tensor_add(dst[:, t, :], tmp2[:, :], b_bc)

        layernorm(h_sb, x1_tok, g1_bc, b1_bc, p4_tmp, p4_stat)

        # transpose x1 to feature-major bf16 for the FF matmuls
        p4_tp = tc.alloc_tile_pool(name="p4_tp", bufs=2, space="PSUM")
        for et in range(ET):
            ps = p4_tp.tile([P, 512], F32, tag="x1tp", name="x1tp")
            for qt in range(TO):
                nc.tensor.transpose(
                    ps[:, qt * P : (qt + 1) * P],
                    x1_tok[:, qt, et * P : (et + 1) * P],
                    id_f32[:, :],
                )
            nc.scalar.activation(x1_T[:, et, :], ps[:, :], AF.Copy)
        p4_tp.release()
        p4_psum.release()
        p4_stat.release()
        p4_tmp.release()
        p4_w.release()
        p_ctx.release()
        p_xog.release()

        # ================= P5a: FF1 / FFg + gating ==========================
        p_ffT = tc.alloc_tile_pool(name="p_ffT", bufs=1, side="right")
        ff_T = p_ffT.tile([P, FT, T_OWN], BF16)
        p5_w = tc.alloc_tile_pool(name="p5_w", bufs=2)
        p5_sg = tc.alloc_tile_pool(name="p5_sg", bufs=3)
        p5_psum = tc.alloc_tile_pool(name="p5_psum", bufs=2, space="PSUM")
        for jg in range(4):
            wf1_sb = p5_w.tile([P, ET, 1024], BF16, tag="wf1", name="wf1s")
            wfg_sb = p5_w.tile([P, ET, 1024], BF16, tag="wfg", name="wfgs")
            for et in range(ET):
                nc.sync.dma_start(
                    out=wf1_sb[:, et, :],
                    in_=wf1_d[et * P : (et + 1) * P, jg * 1024 : (jg + 1) * 1024],
                )
                nc.sync.dma_start(
                    out=wfg_sb[:, et, :],
                    in_=wfg_d[et * P : (et + 1) * P, jg * 1024 : (jg + 1) * 1024],
                )
            for jj in range(8):
                j = jg * 8 + jj
                psa = p5_psum.tile([P, 512], F32, tag="ffa", name="ffa")
                psb = p5_psum.tile([P, 512], F32, tag="ffb", name="ffb")
                for et in range(ET):
                    nc.tensor.matmul(
                        psa[:, :],
                        lhsT=wf1_sb[:, et, jj * P : (jj + 1) * P],
                        rhs=x1_T[:, et, :],
                        start=(et == 0),
                        stop=(et == ET - 1),
                    )
                for et in range(ET):
                    nc.tensor.matmul(
                        psb[:, :],
                        lhsT=wfg_sb[:, et, jj * P : (jj + 1) * P],
                        rhs=x1_T[:, et, :],
                        start=(et == 0),
                        stop=(et == ET - 1),
                    )
                sg = p5_sg.tile([P, 512], BF16, tag="sg", name="sg")
                nc.scalar.activation(
                    sg[:, :], psb[:, :], AF.Sigmoid, bias=bfg_sb[:, j : j + 1]
                )
                nc.vector.scalar_tensor_tensor(
                    out=ff_T[:, j, :],
                    in0=psa[:, :],
                    scalar=bf1_sb[:, j : j + 1],
                    in1=sg[:, :],
                    op0=OP.add,
                    op1=OP.mult,
                )
        p5_psum.release()
        p5_sg.release()
        p5_w.release()
        p_x1T.release()

        # ================= P5b: FF2 + residual + LN2 ========================
        p6_w = tc.alloc_tile_pool(name="p6_w", bufs=3)
        p6_tmp = tc.alloc_tile_pool(name="p6_tmp", bufs=2)
        p6_stat = tc.alloc_tile_pool(name="p6_stat", bufs=4)
        p6_out = tc.alloc_tile_pool(name="p6_out", bufs=1)
        p6_psum = tc.alloc_tile_pool(name="p6_psum", bufs=1, space="PSUM")
        out_sb = p6_out.tile([P, TO, E], F32)
        f2ps = [p6_psum.tile([P, 512], F32, tag=f"f2_{i}", name=f"f2_{i}") for i in range(8)]
        for j in range(FT):
            wf2_sb = p6_w.tile([P, E], BF16, tag="wf2", name="wf2s")
            nc.sync.dma_start(out=wf2_sb[:, :], in_=wf2_d[j * P : (j + 1) * P, :])
            for t in range(TO):
                for eg in range(2):
                    nc.tensor.matmul(
                        f2ps[t * 2 + eg][:, :],
                        lhsT=ff_T[:, j, t * P : (t + 1) * P],
                        rhs=wf2_sb[:, eg * 512 : (eg + 1) * 512],
                        start=(j == 0),
                        stop=False,
                    )
        for t in range(TO):
            for eg in range(2):
                nc.tensor.matmul(
                    f2ps[t * 2 + eg][:, :],
                    lhsT=ones_row[:, :],
                    rhs=bf2_row[:, eg * 512 : (eg + 1) * 512],
                    start=False,
                    stop=True,
                )
                nc.vector.tensor_add(
                    h_sb[:, t, eg * 512 : (eg + 1) * 512],
                    f2ps[t * 2 + eg][:, :],
                    x1_tok[:, t, eg * 512 : (eg + 1) * 512],
                )

        layernorm(h_sb, out_sb, g2_bc, b2_bc, p6_tmp, p6_stat)
        for t in range(TO):
            nc.sync.dma_start(out=out_d[t * P : (t + 1) * P, :], in_=out_sb[:, t, :])
        p6_psum.release()
        p6_out.release()
        p6_stat.release()
        p6_tmp.release()
        p6_w.release()
        p_ffT.release()
        p_h.release()
        always.release()

    nc.compile()
    return nc


def get_nc():
    if "nc" not in _CACHE:
        _CACHE["nc"] = _build_nc()
    return _CACHE["nc"]


def make_in_maps(inputs):
    import ml_dtypes

    bf16 = ml_dtypes.bfloat16
    x = np.asarray(inputs["x"], np.float32)
    pe = np.asarray(inputs["pos_encoding"], np.float32)
    wcast = {
        k: np.ascontiguousarray(np.asarray(inputs[K], np.float32).astype(bf16))
        for k, K in [
            ("wq", "Wq"), ("wk", "Wk"), ("wv", "Wv"), ("wg", "Wg"), ("wo", "Wo"),
            ("wf1", "Wf1"), ("wfg", "Wfg"), ("wf2", "Wf2"),
        ]
    }
    bias = {
        k: np.ascontiguousarray(np.asarray(inputs[K], np.float32))
        for k, K in [
            ("bq", "bq"), ("bk", "bk"), ("bf1", "bf1"), ("bfg", "bfg"),
            ("g1", "ln1_g"), ("b1", "ln1_b"), ("g2", "ln2_g"), ("b2", "ln2_b"),
        ]
    }
    bias["brows"] = np.ascontiguousarray(
        np.concatenate(
            [np.asarray(inputs[k], np.float32) for k in ["bv", "bg", "bo", "bf2"]]
        ).astype(bf16)
    )
    in_maps = []
    for c in range(N_CORES):
        b, half = divmod(c, 2)
        own = slice(half * T_OWN, (half + 1) * T_OWN)
        oth = slice((1 - half) * T_OWN, (2 - half) * T_OWN)
        x_perm = np.ascontiguousarray(np.concatenate([x[b, own], x[b, oth]], axis=0))
        pos_c = pe[own, :, :]
        if half == 1:
            pos_c = np.concatenate([pos_c[:, own, :], pos_c[:, oth, :]], axis=1)
        pos_c = np.ascontiguousarray(pos_c.reshape(T_OWN, S * 8))
        in_maps.append({"x": x_perm, "pos": pos_c, **wcast, **bias})
    return in_maps


def assemble_output(results):
    out = np.empty((B, S, E), np.float32)
    for c in range(N_CORES):
        b, half = divmod(c, 2)
        out[b, half * T_OWN : (half + 1) * T_OWN] = results[c]["out"]
    return out


def kernel(**inputs):
    from concourse.bass_utils import run_bass_kernel_spmd

    nc = get_nc()
    in_maps = make_in_maps(inputs)
    res = run_bass_kernel_spmd(nc, in_maps, core_ids=list(range(N_CORES)))
    return assemble_output(res.results)


if __name__ == "__main__":
    get_nc()
    print("build+compile OK")


# revision 12
# speedup vs baseline: 1.1289x; 1.1289x over previous
"""Trainium2 Bass kernel for nn_AttentionBlock (B=4, S=1024, E=1024, H=16, FF=4096).

Sharding: 8-way data-parallel over (batch, seq-half) tokens. Each core owns 512
query tokens of one batch; it computes K/V for the batch's full 1024 tokens
locally (cheap redundant compute instead of a collective). All matmuls run in
bf16 on the TensorEngine (f32 PSUM accumulation); softmax, residuals and
layernorms stay in f32.

v2 layout notes:
- pos-bias DMA + DVE 8:1 reduce run first, overlapped with x/QKV work.
- attention uses 2-bank score PSUM tiles so exp runs on [128,1024] tiles.
- softmax 1/sum is batched: one [16,512] reciprocal after the head loop,
  broadcast to [64,512] tiles via K=1 matmuls.
- FF2 runs in two token groups so LN2 + output DMA overlap the second group.

SBUF pool lifetimes are two LIFO stacks (releases must be stack-ordered):
  left : always | p_qkv | p2_tok | p2_stage | p_xT | transients | p_h | p_x1T |...
  right: p_xog(->P4) | p_ctx(P3->P4) || p_ffT(P5a->P5b)
"""

import sys

sys.path.insert(0, "/opt/trn_rl_repo")

import numpy as np

B, S, E = 4, 1024, 1024
H, DK, HS = 16, 64, 64
FF = 4096
EPS = 1e-6
N_CORES = 8
T_OWN = 512  # tokens owned per core
T_FULL = 1024  # tokens of the core's batch (for K/V)
P = 128

_CACHE = {}


def _build_nc():
    import concourse.bass as bass  # noqa: F401
    import concourse.mybir as mybir
    import concourse.tile as tile
    from concourse import bacc
    from concourse.masks import make_identity

    F32 = mybir.dt.float32
    BF16 = mybir.dt.bfloat16
    AX = mybir.AxisListType
    AF = mybir.ActivationFunctionType
    OP = mybir.AluOpType

    nc = bacc.Bacc(None, target_bir_lowering=False, debug=False)

    x_d = nc.declare_dram_parameter("x", [T_FULL, E], F32, isOutput=False)
    pos_d = nc.declare_dram_parameter("pos", [T_OWN, S * 8], F32, isOutput=False)
    wq_d = nc.declare_dram_parameter("wq", [E, H * DK], BF16, isOutput=False)
    wk_d = nc.declare_dram_parameter("wk", [E, H * DK], BF16, isOutput=False)
    wv_d = nc.declare_dram_parameter("wv", [E, H * HS], BF16, isOutput=False)
    wg_d = nc.declare_dram_parameter("wg", [E, E], BF16, isOutput=False)
    wo_d = nc.declare_dram_parameter("wo", [H * HS, E], BF16, isOutput=False)
    wf1_d = nc.declare_dram_parameter("wf1", [E, FF], BF16, isOutput=False)
    wfg_d = nc.declare_dram_parameter("wfg", [E, FF], BF16, isOutput=False)
    wf2_d = nc.declare_dram_parameter("wf2", [FF, E], BF16, isOutput=False)
    bq_d = nc.declare_dram_parameter("bq", [H * DK], F32, isOutput=False)
    bk_d = nc.declare_dram_parameter("bk", [H * DK], F32, isOutput=False)
    brows_d = nc.declare_dram_parameter("brows", [4 * E], BF16, isOutput=False)
    bf1_d = nc.declare_dram_parameter("bf1", [FF], F32, isOutput=False)
    bfg_d = nc.declare_dram_parameter("bfg", [FF], F32, isOutput=False)
    g1_d = nc.declare_dram_parameter("g1", [E], F32, isOutput=False)
    b1_d = nc.declare_dram_parameter("b1", [E], F32, isOutput=False)
    g2_d = nc.declare_dram_parameter("g2", [E], F32, isOutput=False)
    b2_d = nc.declare_dram_parameter("b2", [E], F32, isOutput=False)
    out_d = nc.declare_dram_parameter("out", [T_OWN, E], F32, isOutput=True)

    ET = E // P  # 8 e-tiles
    TT = T_FULL // P  # 8 token tiles (full)
    TO = T_OWN // P  # 4 own token tiles
    FT = FF // P  # 32 ff tiles

    with tile.TileContext(nc) as tc:
        # ---- whole-kernel pool: constants, biases ---------------------------
        always = tc.alloc_tile_pool(name="always", bufs=1)

        id_f32 = always.tile([P, P], F32)
        make_identity(nc, id_f32)
        id_bf = always.tile([P, P], BF16)
        make_identity(nc, id_bf)
        ones_row = always.tile([1, P], BF16)
        nc.gpsimd.memset(ones_row[:, :], 1.0)
        eps_col = always.tile([P, 1], F32)
        nc.gpsimd.memset(eps_col[:, :], EPS)

        bq_sb = always.tile([P, ET], F32)
        nc.sync.dma_start(out=bq_sb[:, :], in_=bq_d[:].rearrange("(a p) -> p a", p=P))
        bk_sb = always.tile([P, ET], F32)
        nc.sync.dma_start(out=bk_sb[:, :], in_=bk_d[:].rearrange("(a p) -> p a", p=P))
        bf1_sb = always.tile([P, FT], F32)
        nc.sync.dma_start(out=bf1_sb[:, :], in_=bf1_d[:].rearrange("(a p) -> p a", p=P))
        bfg_sb = always.tile([P, FT], F32)
        nc.sync.dma_start(out=bfg_sb[:, :], in_=bfg_d[:].rearrange("(a p) -> p a", p=P))

        brow_bf = always.tile([1, 4 * E], BF16)
        nc.sync.dma_start(out=brow_bf[:, :], in_=brows_d[None, :])
        bv_row = brow_bf[:, 0:E]
        bg_row = brow_bf[:, E : 2 * E]
        bo_row = brow_bf[:, 2 * E : 3 * E]
        bf2_row = brow_bf[:, 3 * E : 4 * E]

        ln_bc = always.tile([P, 4 * E], F32)
        ones_col_f32 = always.tile([1, P], F32)
        nc.gpsimd.memset(ones_col_f32[:, :], 1.0)
        p_lnrow = tc.alloc_tile_pool(name="p_lnrow", bufs=1)
        p_lnb_psum = tc.alloc_tile_pool(name="p_lnb_psum", bufs=2, space="PSUM")
        ln_row = p_lnrow.tile([1, 4 * E], F32)
        nc.sync.dma_start(out=ln_row[:, 0:E], in_=g1_d[None, :])
        nc.sync.dma_start(out=ln_row[:, E : 2 * E], in_=b1_d[None, :])
        nc.sync.dma_start(out=ln_row[:, 2 * E : 3 * E], in_=g2_d[None, :])
        nc.sync.dma_start(out=ln_row[:, 3 * E : 4 * E], in_=b2_d[None, :])
        # broadcast ln rows to all partitions via K=1 matmul
        for c8 in range(8):
            psb = p_lnb_psum.tile([P, 512], F32, tag="lnb", name="lnb")
            nc.tensor.matmul(
                psb[:, :],
                lhsT=ones_col_f32[:, :],
                rhs=ln_row[:, c8 * 512 : (c8 + 1) * 512],
                start=True,
                stop=True,
            )
            nc.scalar.activation(ln_bc[:, c8 * 512 : (c8 + 1) * 512], psb[:, :], AF.Copy)
        p_lnb_psum.release()
        p_lnrow.release()
        g1_bc = ln_bc[:, 0:E]
        b1_bc = ln_bc[:, E : 2 * E]
        g2_bc = ln_bc[:, 2 * E : 3 * E]
        b2_bc = ln_bc[:, 3 * E : 4 * E]

        # ---- persistent activations ----------------------------------------
        p_xog = tc.alloc_tile_pool(name="p_xog", bufs=1, side="right")
        x_own = p_xog.tile([P, TO, E], F32)
        gate_sb = p_xog.tile([P, TO, E], F32)

        p_qkv = tc.alloc_tile_pool(name="p_qkv", bufs=1)
        q_T = p_qkv.tile([P, ET, T_OWN], BF16)
        k_T = p_qkv.tile([P, ET, T_FULL], BF16)
        v_aug = p_qkv.tile([P, TT, H * (HS + 1)], BF16)
        pos_T = p_qkv.tile([P, TT, T_OWN], BF16)
        v_aug4 = v_aug.rearrange("p t (h c) -> p t h c", c=HS + 1)

        # ================= P2a: pos DMA + 8:1 reduce (overlaps P0/P1) ======
        p2_tok = tc.alloc_tile_pool(name="p2_tok", bufs=1)
        p2_stage = tc.alloc_tile_pool(name="p2_stage", bufs=3)
        pos_tok = p2_tok.tile([P, TO, S], F32)
        for qt in range(TO):
            for kc in range(4):
                stage = p2_stage.tile([P, 2048], F32, tag="pos", name="poss")
                nc.sync.dma_start(
                    out=stage[:, :],
                    in_=pos_d[qt * P : (qt + 1) * P, kc * 2048 : (kc + 1) * 2048],
                )
                nc.vector.tensor_reduce(
                    pos_tok[:, qt, kc * 256 : (kc + 1) * 256],
                    stage.rearrange("p (k c) -> p k c", c=8),
                    axis=AX.X,
                    op=OP.add,
                )

        p_xT = tc.alloc_tile_pool(name="p_xT", bufs=1)
        x_T = p_xT.tile([P, ET, T_FULL], BF16)

        nc.sync.dma_start(
            out=x_own[:, :, :],
            in_=x_d[0:T_OWN, :].rearrange("(t p) e -> p t e", p=P),
        )

        # ================= P0: load x, transpose to feature-major ==========
        p0_stage = tc.alloc_tile_pool(name="p0_stage", bufs=1)
        p0_psum = tc.alloc_tile_pool(name="p0_psum", bufs=2, space="PSUM")
        x_stage = p0_stage.tile([P, TT, E], F32)
        for t in range(TT):
            nc.sync.dma_start(out=x_stage[:, t, :], in_=x_d[t * P : (t + 1) * P, :])
        for et in range(ET):
            for tg in range(2):
                ps = p0_psum.tile([P, 512], F32, tag="tp", name="tp")
                for i in range(4):
                    t = tg * 4 + i
                    nc.tensor.transpose(
                        ps[:, i * P : (i + 1) * P],
                        x_stage[:, t, et * P : (et + 1) * P],
                        id_f32[:, :],
                    )
                nc.scalar.activation(
                    x_T[:, et, tg * 512 : (tg + 1) * 512], ps[:, :], AF.Copy
                )
        p0_psum.release()
        p0_stage.release()

        # ================= P1: QKV + gate projections =======================
        p1_w = tc.alloc_tile_pool(name="p1_w", bufs=2)
        p1_psum = tc.alloc_tile_pool(name="p1_psum", bufs=4, space="PSUM")
        wq_sb = p1_w.tile([P, ET, H * DK], BF16, tag="w", name="wq_sb")
        wk_sb = p1_w.tile([P, ET, H * DK], BF16, tag="w", name="wk_sb")
        wv_sb = p1_w.tile([P, ET, H * HS], BF16, tag="w", name="wv_sb")
        wg_sb = p1_w.tile([P, ET, E], BF16, tag="w", name="wg_sb")
        for et in range(ET):
            nc.sync.dma_start(out=wq_sb[:, et, :], in_=wq_d[et * P : (et + 1) * P, :])
        for et in range(ET):
            nc.sync.dma_start(out=wk_sb[:, et, :], in_=wk_d[et * P : (et + 1) * P, :])
        for et in range(ET):
            nc.sync.dma_start(out=wv_sb[:, et, :], in_=wv_d[et * P : (et + 1) * P, :])
        for et in range(ET):
            nc.sync.dma_start(out=wg_sb[:, et, :], in_=wg_d[et * P : (et + 1) * P, :])

        # q_T[nt] = (x[own] @ Wq)_T   (feature-major out via lhsT=W)
        for nt in range(ET):
            ps = p1_psum.tile([P, 512], F32, tag="qk", name="psq")
            for et in range(ET):
                nc.tensor.matmul(
                    ps[:, :],
                    lhsT=wq_sb[:, et, nt * P : (nt + 1) * P],
                    rhs=x_T[:, et, 0:T_OWN],
                    start=(et == 0),
                    stop=(et == ET - 1),
                )
            nc.scalar.activation(
                q_T[:, nt, :], ps[:, :], AF.Identity, bias=bq_sb[:, nt : nt + 1]
            )
        # k_T[nt] over full 1024 tokens
        for nt in range(ET):
            for ch in range(2):
                ps = p1_psum.tile([P, 512], F32, tag="qk", name="psk")
                for et in range(ET):
                    nc.tensor.matmul(
                        ps[:, :],
                        lhsT=wk_sb[:, et, nt * P : (nt + 1) * P],
                        rhs=x_T[:, et, ch * 512 : (ch + 1) * 512],
                        start=(et == 0),
                        stop=(et == ET - 1),
                    )
                nc.scalar.activation(
                    k_T[:, nt, ch * 512 : (ch + 1) * 512],
                    ps[:, :],
                    AF.Identity,
                    bias=bk_sb[:, nt : nt + 1],
                )
        # v token-major over full tokens (+ bias row); scatter per head
        for t in range(TT):
            for hg in range(2):
                ps = p1_psum.tile([P, 512], F32, tag="qk", name="psv")
                for et in range(ET):
                    nc.tensor.matmul(
                        ps[:, :],
                        lhsT=x_T[:, et, t * P : (t + 1) * P],
                        rhs=wv_sb[:, et, hg * 512 : (hg + 1) * 512],
                        start=(et == 0),
                        stop=False,
                    )
                nc.tensor.matmul(
                    ps[:, :],
                    lhsT=ones_row[:, :],
                    rhs=bv_row[:, hg * 512 : (hg + 1) * 512],
                    start=False,
                    stop=True,
                )
                nc.vector.tensor_copy(
                    v_aug4[:, t, hg * 8 : (hg + 1) * 8, 0:HS],
                    ps[:, :].rearrange("p (h c) -> p h c", c=HS),
                )
        nc.gpsimd.memset(v_aug4[:, :, :, HS : HS + 1], 1.0)
        # gate = sigmoid(x @ Wg + bg), token-major, own tokens
        for t in range(TO):
            for eg in range(2):
                ps = p1_psum.tile([P, 512], F32, tag="qk", name="psg")
                for et in range(ET):
                    nc.tensor.matmul(
                        ps[:, :],
                        lhsT=x_T[:, et, t * P : (t + 1) * P],
                        rhs=wg_sb[:, et, eg * 512 : (eg + 1) * 512],
                        start=(et == 0),
                        stop=False,
                    )
                nc.tensor.matmul(
                    ps[:, :],
                    lhsT=ones_row[:, :],
                    rhs=bg_row[:, eg * 512 : (eg + 1) * 512],
                    start=False,
                    stop=True,
                )
                nc.scalar.activation(
                    gate_sb[:, t, eg * 512 : (eg + 1) * 512], ps[:, :], AF.Sigmoid
                )
        p1_psum.release()
        p1_w.release()
        p_xT.release()
        p2_stage.release()

        # ================= P2b: pos transposes to [k, q] ====================
        p2_psum = tc.alloc_tile_pool(name="p2_psum", bufs=2, space="PSUM")
        for kt in range(TT):
            ps = p2_psum.tile([P, 512], F32, tag="ptp", name="ptps")
            for qt in range(TO):
                nc.tensor.transpose(
                    ps[:, qt * P : (qt + 1) * P],
                    pos_tok[:, qt, kt * P : (kt + 1) * P],
                    id_f32[:, :],
                )
            nc.scalar.activation(pos_T[:, kt, :], ps[:, :], AF.Copy)
        p2_psum.release()
        p2_tok.release()

        # ================= P3: attention ====================================
        p_ctx = tc.alloc_tile_pool(name="p_ctx", bufs=1, side="right")
        ctx_T = p_ctx.tile([P, ET, T_OWN], BF16)
        p3_misc = tc.alloc_tile_pool(name="p3_misc", bufs=1)
        sums_sb = p3_misc.tile([H, T_OWN], F32)
        r16 = p3_misc.tile([H, T_OWN], F32)
        r16b = p3_misc.tile([H, T_OWN], BF16)
        r_row = p3_misc.tile([1, H, T_OWN], BF16)
        p3_ctxu = tc.alloc_tile_pool(name="p3_ctxu", bufs=1)
        ctxu = p3_ctxu.tile([P, ET, T_OWN], F32)
        p3_srow = tc.alloc_tile_pool(name="p3_srow", bufs=3)
        p3_p = tc.alloc_tile_pool(name="p3_p", bufs=2)
        p3_sc = tc.alloc_tile_pool(name="p3_sc", bufs=2, space="PSUM")
        p3_pv = tc.alloc_tile_pool(name="p3_pv", bufs=3, space="PSUM")
        for h in range(H):
            hp = h % 2
            nt = h // 2
            pt = p3_p.tile([P, TT, 512], BF16, tag="pt", name="pt")
            pv = p3_pv.tile([P, 512], F32, tag="pv", name="pv")
            for kp in range(TT // 2):
                sc = p3_sc.tile([P, 2, 512], F32, tag="sc", name="sc")
                for j in range(2):
                    kt = 2 * kp + j
                    nc.tensor.matmul(
                        sc[:, j, :],
                        lhsT=k_T[hp * 64 : hp * 64 + 64, nt, kt * P : (kt + 1) * P],
                        rhs=q_T[hp * 64 : hp * 64 + 64, nt, :],
                        start=True,
                        stop=False,
                    )
                    nc.tensor.matmul(
                        sc[:, j, :],
                        lhsT=id_bf[:, :],
                        rhs=pos_T[:, kt, :],
                        start=False,
                        stop=True,
                    )
                nc.scalar.activation(
                    pt[:, 2 * kp : 2 * kp + 2, :], sc[:, :, :], AF.Exp, scale=0.125
                )
                for j in range(2):
                    kt = 2 * kp + j
                    nc.tensor.matmul(
                        pv[0 : HS + 1, :],
                        lhsT=v_aug4[:, kt, h, :],
                        rhs=pt[:, kt, :],
                        start=(kt == 0),
                        stop=(kt == TT - 1),
                    )
            # stash unnormalized ctx + the softmax sums; normalize after all heads
            nc.vector.tensor_copy(ctxu[hp * 64 : hp * 64 + 64, nt, :], pv[0:HS, :])
            srow = p3_srow.tile([1, T_OWN], F32, tag="srow", name="srow")
            nc.vector.tensor_copy(srow[:, :], pv[HS : HS + 1, :])
            nc.sync.dma_start(out=sums_sb[h : h + 1, :], in_=srow[:, :])
        p3_pv.release()
        p3_sc.release()
        p3_p.release()
        p3_srow.release()

        # batched softmax normalization
        nc.vector.reciprocal(r16[:, :], sums_sb[:, :])
        nc.vector.tensor_copy(r16b[:, :], r16[:, :])
        for h in range(H):
            nc.sync.dma_start(out=r_row[:, h, :], in_=r16b[h : h + 1, :])
        p3_rf = tc.alloc_tile_pool(name="p3_rf", bufs=2, space="PSUM")
        p3_rfsb = tc.alloc_tile_pool(name="p3_rfsb", bufs=2)
        for nt in range(ET):
            rf = p3_rf.tile([P, 512], F32, tag="rf", name="rf")
            nc.tensor.matmul(
                rf[0:64, :],
                lhsT=ones_row[:, 0:64],
                rhs=r_row[:, 2 * nt, :],
                start=True,
                stop=True,
            )
            nc.tensor.matmul(
                rf[64:128, :],
                lhsT=ones_row[:, 0:64],
                rhs=r_row[:, 2 * nt + 1, :],
                start=True,
                stop=True,
            )
            rf_sb = p3_rfsb.tile([P, 512], F32, tag="rfsb", name="rfsb")
            nc.scalar.activation(rf_sb[:, :], rf[:, :], AF.Copy)
            nc.vector.tensor_mul(ctx_T[:, nt, :], ctxu[:, nt, :], rf_sb[:, :])
        p3_rfsb.release()
        p3_rf.release()
        p3_ctxu.release()
        p3_misc.release()
        p_qkv.release()

        # ================= P4: Wo + gating + LN1 ============================
        p_h = tc.alloc_tile_pool(name="p_h", bufs=1)
        h_sb = p_h.tile([P, TO, E], F32)
        x1_tok = p_h.tile([P, TO, E], F32)
        p_x1T = tc.alloc_tile_pool(name="p_x1T", bufs=1)
        x1_T = p_x1T.tile([P, ET, T_OWN], BF16)
        p4_w = tc.alloc_tile_pool(name="p4_w", bufs=1)
        p4_tmp = tc.alloc_tile_pool(name="p4_tmp", bufs=2)
        p4_stat = tc.alloc_tile_pool(name="p4_stat", bufs=16)
        p4_psum = tc.alloc_tile_pool(name="p4_psum", bufs=2, space="PSUM")
        wo_sb = p4_w.tile([P, ET, E], BF16)
        for et in range(ET):
            nc.sync.dma_start(out=wo_sb[:, et, :], in_=wo_d[et * P : (et + 1) * P, :])
        for t in range(TO):
            for eg in range(2):
                ps = p4_psum.tile([P, 512], F32, tag="o", name="pso")
                for hst in range(ET):
                    nc.tensor.matmul(
                        ps[:, :],
                        lhsT=ctx_T[:, hst, t * P : (t + 1) * P],
                        rhs=wo_sb[:, hst, eg * 512 : (eg + 1) * 512],
                        start=(hst == 0),
                        stop=False,
                    )
                nc.tensor.matmul(
                    ps[:, :],
                    lhsT=ones_row[:, :],
                    rhs=bo_row[:, eg * 512 : (eg + 1) * 512],
                    start=False,
                    stop=True,
                )
                tmp = p4_tmp.tile([P, 512], F32, tag="gat", name="gat")
                nc.vector.tensor_mul(
                    tmp[:, :], ps[:, :], gate_sb[:, t, eg * 512 : (eg + 1) * 512]
                )
                nc.vector.tensor_add(
                    h_sb[:, t, eg * 512 : (eg + 1) * 512],
                    tmp[:, :],
                    x_own[:, t, eg * 512 : (eg + 1) * 512],
                )

        def layernorm(src, dst, g_bc, b_bc, tmpp, statp, trange):
            # dst = (src - mean) / sqrt(var + eps) * g + b, rows = tokens.
            # Two sweeps so all Ln's batch together and all Exp's batch
            # together (one ACT table set).
            stats = {}
            for t in trange:
                scr = tmpp.tile([P, E], F32, tag="scr", name="scr")
                ssum = statp.tile([P, 1], F32, tag="st", name="ssum")
                nc.scalar.activation(
                    scr[:, :], src[:, t, :], AF.Identity, accum_out=ssum[:, :]
                )
                scr2 = tmpp.tile([P, E], F32, tag="scr2", name="scr2")
                ssq = statp.tile([P, 1], F32, tag="st", name="ssq")
                nc.scalar.activation(
                    scr2[:, :], src[:, t, :], AF.Square, accum_out=ssq[:, :]
                )
                nmu = statp.tile([P, 1], F32, tag="st", name="nmu")
                nc.vector.tensor_scalar_mul(nmu[:, :], ssum[:, :], -1.0 / E)
                musq = statp.tile([P, 1], F32, tag="st", name="musq")
                nc.vector.tensor_mul(musq[:, :], nmu[:, :], nmu[:, :])
                var = statp.tile([P, 1], F32, tag="st", name="var")
                nc.vector.scalar_tensor_tensor(
                    out=var[:, :],
                    in0=ssq[:, :],
                    scalar=1.0 / E,
                    in1=musq[:, :],
                    op0=OP.mult,
                    op1=OP.subtract,
                )
                lnv = statp.tile([P, 1], F32, tag="st", name="lnv")
                nc.scalar.activation(lnv[:, :], var[:, :], AF.Ln, bias=eps_col[:, :])
                stats[t] = (nmu, lnv)
            for t in trange:
                nmu, lnv = stats[t]
                # rstd = exp(-0.5 * ln(var + eps))
                rstd = statp.tile([P, 1], F32, tag="st", name="rstd")
                nc.scalar.activation(rstd[:, :], lnv[:, :], AF.Exp, scale=-0.5)
                bprime = statp.tile([P, 1], F32, tag="st", name="bprime")
                nc.vector.tensor_mul(bprime[:, :], nmu[:, :], rstd[:, :])
                xc = tmpp.tile([P, E], F32, tag="xc", name="xc")
                nc.scalar.activation(
                    xc[:, :],
                    src[:, t, :],
                    AF.Identity,
                    bias=bprime[:, :],
                    scale=rstd[:, :],
                )
                tmp2 = tmpp.tile([P, E], F32, tag="xg", name="xg")
                nc.vector.tensor_mul(tmp2[:, :], xc[:, :], g_bc)
                nc.vector.tensor_add(dst[:, t, :], tmp2[:, :], b_bc)

        layernorm(h_sb, x1_tok, g1_bc, b1_bc, p4_tmp, p4_stat, range(TO))

        # transpose x1 to feature-major bf16 for the FF matmuls
        p4_tp = tc.alloc_tile_pool(name="p4_tp", bufs=2, space="PSUM")
        for et in range(ET):
            ps = p4_tp.tile([P, 512], F32, tag="x1tp", name="x1tp")
            for qt in range(TO):
                nc.tensor.transpose(
                    ps[:, qt * P : (qt + 1) * P],
                    x1_tok[:, qt, et * P : (et + 1) * P],
                    id_f32[:, :],
                )
            nc.scalar.activation(x1_T[:, et, :], ps[:, :], AF.Copy)
        p4_tp.release()
        p4_psum.release()
        p4_stat.release()
        p4_tmp.release()
        p4_w.release()
        p_ctx.release()
        p_xog.release()

        # ================= P5a: FF1 / FFg + gating ==========================
        p_ffT = tc.alloc_tile_pool(name="p_ffT", bufs=1, side="right")
        ff_T = p_ffT.tile([P, FT, T_OWN], BF16)
        p5_w = tc.alloc_tile_pool(name="p5_w", bufs=2)
        p5_sg = tc.alloc_tile_pool(name="p5_sg", bufs=3)
        p5_psum = tc.alloc_tile_pool(name="p5_psum", bufs=2, space="PSUM")
        for jg in range(4):
            wf1_sb = p5_w.tile([P, ET, 1024], BF16, tag="wf1", name="wf1s")
            wfg_sb = p5_w.tile([P, ET, 1024], BF16, tag="wfg", name="wfgs")
            for et in range(ET):
                nc.sync.dma_start(
                    out=wf1_sb[:, et, :],
                    in_=wf1_d[et * P : (et + 1) * P, jg * 1024 : (jg + 1) * 1024],
                )
                nc.sync.dma_start(
                    out=wfg_sb[:, et, :],
                    in_=wfg_d[et * P : (et + 1) * P, jg * 1024 : (jg + 1) * 1024],
                )
            for jj in range(8):
                j = jg * 8 + jj
                psa = p5_psum.tile([P, 512], F32, tag="ffa", name="ffa")
                psb = p5_psum.tile([P, 512], F32, tag="ffb", name="ffb")
                for et in range(ET):
                    nc.tensor.matmul(
                        psa[:, :],
                        lhsT=wf1_sb[:, et, jj * P : (jj + 1) * P],
                        rhs=x1_T[:, et, :],
                        start=(et == 0),
                        stop=(et == ET - 1),
                    )
                for et in range(ET):
                    nc.tensor.matmul(
                        psb[:, :],
                        lhsT=wfg_sb[:, et, jj * P : (jj + 1) * P],
                        rhs=x1_T[:, et, :],
                        start=(et == 0),
                        stop=(et == ET - 1),
                    )
                sg = p5_sg.tile([P, 512], BF16, tag="sg", name="sg")
                nc.scalar.activation(
                    sg[:, :], psb[:, :], AF.Sigmoid, bias=bfg_sb[:, j : j + 1]
                )
                nc.vector.scalar_tensor_tensor(
                    out=ff_T[:, j, :],
                    in0=psa[:, :],
                    scalar=bf1_sb[:, j : j + 1],
                    in1=sg[:, :],
                    op0=OP.add,
                    op1=OP.mult,
                )
        p5_psum.release()
        p5_sg.release()
        p5_w.release()
        p_x1T.release()

        # ================= P5b: FF2 + residual + LN2, two token groups ======
        p6_w = tc.alloc_tile_pool(name="p6_w", bufs=3)
        p6_tmp = tc.alloc_tile_pool(name="p6_tmp", bufs=2)
        p6_stat = tc.alloc_tile_pool(name="p6_stat", bufs=16)
        p6_out = tc.alloc_tile_pool(name="p6_out", bufs=1)
        p6_psum = tc.alloc_tile_pool(name="p6_psum", bufs=1, space="PSUM")
        out_sb = p6_out.tile([P, TO, E], F32)
        for tg in range(2):
            ts_ = [2 * tg, 2 * tg + 1]
            f2ps = [
                p6_psum.tile([P, 512], F32, tag=f"f2_{tg}_{i}", name=f"f2_{tg}_{i}")
                for i in range(4)
            ]
            for j in range(FT):
                wf2_sb = p6_w.tile([P, E], BF16, tag="wf2", name="wf2s")
                nc.sync.dma_start(out=wf2_sb[:, :], in_=wf2_d[j * P : (j + 1) * P, :])
                for i, t in enumerate(ts_):
                    for eg in range(2):
                        nc.tensor.matmul(
                            f2ps[i * 2 + eg][:, :],
                            lhsT=ff_T[:, j, t * P : (t + 1) * P],
                            rhs=wf2_sb[:, eg * 512 : (eg + 1) * 512],
                            start=(j == 0),
                            stop=False,
                        )
            for i, t in enumerate(ts_):
                for eg in range(2):
                    nc.tensor.matmul(
                        f2ps[i * 2 + eg][:, :],
                        lhsT=ones_row[:, :],
                        rhs=bf2_row[:, eg * 512 : (eg + 1) * 512],
                        start=False,
                        stop=True,
                    )
                    nc.vector.tensor_add(
                        h_sb[:, t, eg * 512 : (eg + 1) * 512],
                        f2ps[i * 2 + eg][:, :],
                        x1_tok[:, t, eg * 512 : (eg + 1) * 512],
                    )
            layernorm(h_sb, out_sb, g2_bc, b2_bc, p6_tmp, p6_stat, ts_)
            for t in ts_:
                nc.sync.dma_start(out=out_d[t * P : (t + 1) * P, :], in_=out_sb[:, t, :])
        p6_psum.release()
        p6_out.release()
        p6_stat.release()
        p6_tmp.release()
        p6_w.release()
        p_ffT.release()
        p_h.release()
        always.release()

    nc.compile()
    return nc


def get_nc():
    if "nc" not in _CACHE:
        _CACHE["nc"] = _build_nc()
    return _CACHE["nc"]


def make_in_maps(inputs):
    import ml_dtypes

    bf16 = ml_dtypes.bfloat16
    x = np.asarray(inputs["x"], np.float32)
    pe = np.asarray(inputs["pos_encoding"], np.float32)
    wcast = {
        k: np.ascontiguousarray(np.asarray(inputs[K], np.float32).astype(bf16))
        for k, K in [
            ("wq", "Wq"), ("wk", "Wk"), ("wv", "Wv"), ("wg", "Wg"), ("wo", "Wo"),
            ("wf1", "Wf1"), ("wfg", "Wfg"), ("wf2", "Wf2"),
        ]
    }
    bias = {
        k: np.ascontiguousarray(np.asarray(inputs[K], np.float32))
        for k, K in [
            ("bq", "bq"), ("bk", "bk"), ("bf1", "bf1"), ("bfg", "bfg"),
            ("g1", "ln1_g"), ("b1", "ln1_b"), ("g2", "ln2_g"), ("b2", "ln2_b"),
        ]
    }
    bias["brows"] = np.ascontiguousarray(
        np.concatenate(
            [np.asarray(inputs[k], np.float32) for k in ["bv", "bg", "bo", "bf2"]]
        ).astype(bf16)
    )
    in_maps = []
    for c in range(N_CORES):
        b, half = divmod(c, 2)
        own = slice(half * T_OWN, (half + 1) * T_OWN)
        oth = slice((1 - half) * T_OWN, (2 - half) * T_OWN)
        x_perm = np.ascontiguousarray(np.concatenate([x[b, own], x[b, oth]], axis=0))
        pos_c = pe[own, :, :]
        if half == 1:
            pos_c = np.concatenate([pos_c[:, own, :], pos_c[:, oth, :]], axis=1)
        pos_c = np.ascontiguousarray(pos_c.reshape(T_OWN, S * 8))
        in_maps.append({"x": x_perm, "pos": pos_c, **wcast, **bias})
    return in_maps


def assemble_output(results):
    out = np.empty((B, S, E), np.float32)
    for c in range(N_CORES):
        b, half = divmod(c, 2)
        out[b, half * T_OWN : (half + 1) * T_OWN] = results[c]["out"]
    return out


def kernel(**inputs):
    from concourse.bass_utils import run_bass_kernel_spmd

    nc = get_nc()
    in_maps = make_in_maps(inputs)
    res = run_bass_kernel_spmd(nc, in_maps, core_ids=list(range(N_CORES)))
    return assemble_output(res.results)


if __name__ == "__main__":
    get_nc()
    print("build+compile OK")
